# revision 1
# baseline (speedup 1.0000x reference)
"""Multi-head causal attention (B=4, S=2048, H=1024, 16 heads) on 8 TRN2 cores.

Sharding: batch (4) x head-group (2x8 heads) -> 8 cores. Each core computes,
for one batch and 8 heads: QKV projections, causal softmax attention, and its
partial output projection. Host sums the two head-group partials per batch and
adds the output bias.

Device layout (per core, all matmul operands bf16, fp32 accumulation):
  qT/kT: [512, 2048] (head-major transposed projections), stored as 4
         "pair" tiles [128, 2048] (two 64-dim heads per tile) so that
         scoresT = kT.T @ qT runs as row-tiled K=64 matmul pairs.
  v:     [2048, 512] natural, tiles [128(s), 512(o)].
  scoresT blocks [128(k), 512(q)] in PSUM -> exp on ScalarE -> probsT bf16.
  out.T accumulated per head pair in PSUM via col-tiled M=64 matmul pairs;
  softmax denominators via ones-vector matmuls (M=1) at col offsets 0/32.
  Normalization: exact DVE reciprocal of the denominator rows, expanded
  across partitions with gpsimd partition_broadcast (+ one cross-partition
  copy), then one tensor_tensor multiply per unit.
  Final projection y = out @ WoT accumulates over the 4 pairs.

  Measured on 8 axon TRN2 cores: ~564 us HW exec (max core; mean ~557 us),
  rel err ~0.0059 vs the fp32 reference (bf16 matmul precision). Engine
  balance: ScalarE (exp + per-op semaphore overhead, ~347 us) is the
  serializing resource; PE ~320 us compute; DVE ~190 us.
"""

import sys

sys.path.insert(0, "/opt/trn_rl_repo")

import math
from contextlib import ExitStack

import numpy as np
import ml_dtypes

import concourse.bass as bass
import concourse.mybir as mybir
from concourse import bacc
from concourse.tile import TileContext
from concourse.tile_rust import add_dep_helper
from concourse.bass_utils import run_bass_kernel_spmd

BF16 = mybir.dt.bfloat16
F32 = mybir.dt.float32
AF = mybir.ActivationFunctionType
ALU = mybir.AluOpType

B, S, H = 4, 2048, 1024
NH, DH = 16, 64
O = 512          # per-core output dim of q/k/v projections (8 heads x 64)
NPAIR = 4        # head pairs per core
NSLAB = 4        # q slabs of 512
NST = 16         # s-tiles of 128
MASK_FILL = -8.0e5  # pre-scale (x0.125) additive mask for padded keys

_BUILT = {}


def _build(general_mask: bool):
    if general_mask in _BUILT:
        return _BUILT[general_mask]

    nc = bacc.Bacc("TRN2", target_bir_lowering=False, debug=False)

    xqT = nc.dram_tensor("xqT", [H, S], BF16, kind="ExternalInput")
    xkT = nc.dram_tensor("xkT", [H, S], BF16, kind="ExternalInput")
    xvT = nc.dram_tensor("xvT", [H, S], BF16, kind="ExternalInput")
    wqT = nc.dram_tensor("wqT", [H, O], BF16, kind="ExternalInput")
    wkT = nc.dram_tensor("wkT", [H, O], BF16, kind="ExternalInput")
    wvT = nc.dram_tensor("wvT", [H, O], BF16, kind="ExternalInput")
    woT = nc.dram_tensor("woT", [O, H], BF16, kind="ExternalInput")
    bqc = nc.dram_tensor("bqc", [128, 4], F32, kind="ExternalInput")
    bkc = nc.dram_tensor("bkc", [128, 4], F32, kind="ExternalInput")
    bvr = nc.dram_tensor("bvr", [1, O], BF16, kind="ExternalInput")
    tri = nc.dram_tensor("tri", [128, 896], BF16, kind="ExternalInput")
    mb = nc.dram_tensor("mb", [1, S], BF16, kind="ExternalInput")
    y = nc.dram_tensor("y", [S, H], F32, kind="ExternalOutput")

    with TileContext(nc) as tc, ExitStack() as ctx:
        P = lambda name, bufs, **kw: ctx.enter_context(
            tc.tile_pool(name=name, bufs=bufs, **kw)
        )
        wp = P("wp", 1)
        xt = P("xt", 2)
        qk = P("qk", 1)
        vp = P("vp", 1)
        pb = P("pb", 6)                    # probsT bf16 groups
        ev = P("ev", 3)                    # evacuation temps
        ot = P("ot", 8)                    # outT_sb bf16, held per slab
        ys = P("ys", 4)                    # y sbuf staging
        dp = P("dp", 4, space="DRAM")      # denominators round-trip

        # --- constants / weights ---
        wq_sb = wp.tile([128, 8, O], BF16, tag="wq", name="wq")
        wk_sb = wp.tile([128, 8, O], BF16, tag="wk", name="wk")
        wv_sb = wp.tile([128, 8, O], BF16, tag="wv", name="wv")
        wo_sb = wp.tile([128, 4, H], BF16, tag="wo", name="wo")
        nc.sync.dma_start(wq_sb[:], wqT.rearrange("(po pi) o -> pi po o", pi=128))
        nc.sync.dma_start(wk_sb[:], wkT.rearrange("(po pi) o -> pi po o", pi=128))
        nc.sync.dma_start(wv_sb[:], wvT.rearrange("(po pi) o -> pi po o", pi=128))
        nc.sync.dma_start(wo_sb[:], woT.rearrange("(po pi) j -> pi po j", pi=128))
        bq_sb = wp.tile([128, 4], F32, tag="bq", name="bq")
        bk_sb = wp.tile([128, 4], F32, tag="bk", name="bk")
        bv_sb = wp.tile([1, O], BF16, tag="bv", name="bv")
        tri_sb = wp.tile([128, 896], BF16, tag="tri", name="tri")
        nc.sync.dma_start(bq_sb[:], bqc[:, :])
        nc.sync.dma_start(bk_sb[:], bkc[:, :])
        nc.sync.dma_start(bv_sb[:], bvr[:, :])
        nc.sync.dma_start(tri_sb[:], tri[:, :])
        ones_col = wp.tile([128, 1], BF16, tag="onc", name="onc")
        zeros_col = wp.tile([128, 1], F32, tag="zc", name="zc")
        nc.gpsimd.memset(zeros_col[:], 0.0)
        ones_row = wp.tile([1, 512], BF16, tag="onr", name="onr")
        nc.gpsimd.memset(ones_col[:], 1.0)
        nc.gpsimd.memset(ones_row[:], 1.0)
        if general_mask:
            mb_sb = wp.tile([1, S], BF16, tag="mb", name="mb")
            nc.sync.dma_start(mb_sb[:], mb[:, :])

        # --- projections ---
        qT_sb = [qk.tile([128, S], BF16, tag=f"qT{p}", name=f"qT{p}") for p in range(NPAIR)]
        kT_sb = [qk.tile([128, S], BF16, tag=f"kT{p}", name=f"kT{p}") for p in range(NPAIR)]
        v_sb = vp.tile([128, NST, O], BF16, tag="v", name="v")

        def load_xt(dram):
            t = xt.tile([128, 8, S], BF16, tag="xt", name="xt")
            nc.sync.dma_start(t[:], dram.rearrange("(po pi) s -> pi po s", pi=128))
            return t

        xq_t = load_xt(xqT)
        xk_t = load_xt(xkT)

        def project_qk(x_t, w_sb, b_sb, dst, pp):
            # dst[m][o_in_pair, s] = sum_i w[i, 128m + o] x[i, s] + b
            for m in range(4):
                for sl in range(4):
                    ps = pp.tile([128, 512], F32, tag="pp", name="pp")
                    for ic in range(8):
                        nc.tensor.matmul(
                            ps[:],
                            w_sb[:, ic, 128 * m : 128 * m + 128],
                            x_t[:, ic, 512 * sl : 512 * sl + 512],
                            start=(ic == 0),
                            stop=(ic == 7),
                        )
                    nc.vector.tensor_scalar_add(
                        dst[m][:, 512 * sl : 512 * sl + 512], ps[:], b_sb[:, m : m + 1]
                    )

        with tc.tile_pool(name="pp", bufs=8, space="PSUM") as pp:
            project_qk(xq_t, wq_sb, bq_sb, qT_sb, pp)
            xv_t = load_xt(xvT)
            project_qk(xk_t, wk_sb, bk_sb, kT_sb, pp)

            # v natural: v[s, o] = sum_i x[i, s] w[i, o] + bv[o]
            for st in range(NST):
                ps = pp.tile([128, 512], F32, tag="pp", name="pp")
                for ic in range(8):
                    nc.tensor.matmul(
                        ps[:],
                        xv_t[:, ic, 128 * st : 128 * st + 128],
                        wv_sb[:, ic, :],
                        start=(ic == 0),
                        stop=False,
                    )
                nc.tensor.matmul(
                    ps[:], ones_row[:, 0:128], bv_sb[:, :], start=False, stop=True
                )
                nc.vector.tensor_copy(v_sb[:, st, :], ps[:])

        scp = P("scp", 2, space="PSUM")    # scoresT groups [128,1024]
        otp = P("otp", 1, space="PSUM")    # outT [128,512]
        lp = P("lp", 1, space="PSUM")      # denominators [64,512]
        yp = P("yp", 2, space="PSUM")      # final y [128,512]

        # --- attention + final projection, slab by slab ---
        for slab in range(NSLAB):
            out_sb_tiles = []
            for pair in range(NPAIR):
                n_kt = 4 * (slab + 1)
                q0 = 512 * slab
                ot_ps = otp.tile([128, 512], F32, tag="ot", name="ot")
                l_ps = lp.tile([64, 512], F32, tag="l", name="l")

                def chained_mm(bank, out_ap, lhsT, rhs, start, stop):
                    # HW-verified: start=True clears has_written only for the
                    # written partition slice, so the two col-tiled heads can
                    # run independent accumulation groups in one bank. The
                    # sim's global group-check mis-models partition-offset
                    # outputs, hence skip_group_check.
                    nc.tensor.matmul(
                        out_ap, lhsT, rhs, start=start, stop=stop,
                        skip_group_check=True,
                    )
                for g in range((n_kt + 1) // 2):
                    kts = [kt for kt in (2 * g, 2 * g + 1) if kt < n_kt]
                    used = 512 * len(kts)
                    scA = scp.tile([128, 1024], F32, tag="sc", name="sc")
                    scB = scp.tile([128, 1024], F32, tag="sc", name="sc")
                    for j, kt in enumerate(kts):
                        for hh, sc in ((0, scA), (1, scB)):
                            r0 = 64 * hh
                            nc.tensor.matmul(
                                sc[:, 512 * j : 512 * j + 512],
                                kT_sb[pair][r0 : r0 + 64, 128 * kt : 128 * kt + 128],
                                qT_sb[pair][r0 : r0 + 64, q0 : q0 + 512],
                                start=True,
                                stop=not general_mask,
                            )
                            if general_mask:
                                nc.tensor.matmul(
                                    sc[:, 512 * j : 512 * j + 512],
                                    mb_sb[0:1, 128 * kt : 128 * kt + 128],
                                    ones_row[0:1, :],
                                    start=False,
                                    stop=True,
                                )
                    pbA = pb.tile([128, 1024], BF16, tag="pb", name="pb")
                    pbB = pb.tile([128, 1024], BF16, tag="pb", name="pb")
                    nc.scalar.activation(
                        pbA[:, 0:used], scA[:, 0:used], AF.Exp,
                        bias=zeros_col[:, 0:1], scale=0.125,
                    )
                    nc.scalar.activation(
                        pbB[:, 0:used], scB[:, 0:used], AF.Exp,
                        bias=zeros_col[:, 0:1], scale=0.125,
                    )
                    for j, kt in enumerate(kts):
                        js = slice(512 * j, 512 * j + 512)
                        if kt >= 4 * slab:  # diagonal block: causal triangle
                            o = 128 * (kt - 4 * slab)
                            w = o + 128  # cols >= o+128 are all-ones: skip
                            for p_t in (pbA, pbB):
                                nc.vector.tensor_tensor(
                                    p_t[:, 512 * j : 512 * j + w],
                                    p_t[:, 512 * j : 512 * j + w],
                                    tri_sb[:, 384 - o : 384 - o + w],
                                    ALU.mult,
                                )
                        for hh, p_t in ((0, pbA), (1, pbB)):
                            # one accumulation group per PSUM bank: only the
                            # very first matmul may use start=True (it clears
                            # the whole bank's has_written bits)
                            st = kt == 0
                            sp = kt == n_kt - 1
                            chained_mm(
                                "ot",
                                ot_ps[64 * hh : 64 * hh + 64, :],
                                v_sb[:, kt, 64 * (2 * pair + hh) : 64 * (2 * pair + hh) + 64],
                                p_t[:, js],
                                st,
                                sp,
                            )
                            lr0 = 32 * hh
                            chained_mm(
                                "l",
                                l_ps[lr0 : lr0 + 1, :],
                                ones_col[:, :],
                                p_t[:, js],
                                st,
                                sp,
                            )
                # normalization: recip rows -> DRAM -> broadcast back
                lsb = ev.tile([33, 512], F32, tag="lsb", name="lsb")
                nc.gpsimd.memset(lsb[:], 1.0)
                nc.vector.tensor_copy(lsb[0:1, :], l_ps[0:1, :])
                nc.vector.tensor_copy(lsb[32:33, :], l_ps[32:33, :])
                lrec = ev.tile([33, 512], F32, tag="lrec", name="lrec")
                nc.vector.reciprocal(lrec[:], lsb[:])
                lra = lrec
                lrb = ev.tile([1, 512], F32, tag="lrb", name="lrb")
                nc.vector.tensor_copy(lrb[0:1, :], lrec[32:33, :])
                rx = ev.tile([128, 512], F32, tag="rx", name="rx")
                rxb = ev.tile([128, 512], F32, tag="rxb", name="rxb")
                nc.gpsimd.partition_broadcast(rx[0:128, :], lra[0:1, :])
                nc.gpsimd.partition_broadcast(rxb[0:128, :], lrb[0:1, :])
                nc.vector.tensor_copy(rx[64:128, :], rxb[64:128, :])
                o_sb = ot.tile([128, 512], BF16, tag="ot", name="ot")
                nc.vector.tensor_tensor(o_sb[:], ot_ps[:], rx[:], ALU.mult)
                out_sb_tiles.append(o_sb)

            # final projection for this slab of queries
            for st in range(4):
                srow = 512 * slab + 128 * st
                for jsl in range(2):
                    y_ps = yp.tile([128, 512], F32, tag="yp", name="yp")
                    for pair in range(NPAIR):
                        nc.tensor.matmul(
                            y_ps[:],
                            out_sb_tiles[pair][:, 128 * st : 128 * st + 128],
                            wo_sb[:, pair, 512 * jsl : 512 * jsl + 512],
                            start=(pair == 0),
                            stop=(pair == NPAIR - 1),
                        )
                    ysb = ys.tile([128, 512], F32, tag="ys", name="ys")
                    nc.vector.tensor_copy(ysb[:], y_ps[:])
                    nc.sync.dma_start(
                        y[srow : srow + 128, 512 * jsl : 512 * jsl + 512], ysb[:]
                    )

    nc.compile()
    _BUILT[general_mask] = nc
    return nc


def _prep_core(query, key, value, mask, Wq, bq, Wk, bk, Wv, bv, Wo, core):
    b, hg = core // 2, core % 2
    o_sl = slice(hg * O, hg * O + O)
    bf = ml_dtypes.bfloat16

    tri = np.zeros((128, 896), dtype=np.float32)
    j = np.arange(896)[None, :]
    kk = np.arange(128)[:, None]
    tri[(j - 384) >= kk] = 1.0

    mrow = np.where(mask[b] > 0, 0.0, MASK_FILL).astype(np.float32)

    return {
        "xqT": np.ascontiguousarray(query[b].T).astype(bf),
        "xkT": np.ascontiguousarray(key[b].T).astype(bf),
        "xvT": np.ascontiguousarray(value[b].T).astype(bf),
        "wqT": np.ascontiguousarray(Wq[o_sl].T).astype(bf),
        "wkT": np.ascontiguousarray(Wk[o_sl].T).astype(bf),
        "wvT": np.ascontiguousarray(Wv[o_sl].T).astype(bf),
        "woT": np.ascontiguousarray(Wo[:, o_sl].T).astype(bf),
        "bqc": np.ascontiguousarray(bq[o_sl].reshape(4, 128).T).astype(np.float32),
        "bkc": np.ascontiguousarray(bk[o_sl].reshape(4, 128).T).astype(np.float32),
        "bvr": bv[o_sl].reshape(1, O).astype(bf),
        "tri": tri.astype(bf),
        "mb": mrow.reshape(1, S).astype(bf),
    }


def kernel(query, key, value, mask, Wq, bq, Wk, bk, Wv, bv, Wo, bo, _trace=False):
    general_mask = bool(np.any(np.asarray(mask) <= 0))
    nc = _build(general_mask)
    in_maps = [
        _prep_core(query, key, value, mask, Wq, bq, Wk, bk, Wv, bv, Wo, c)
        for c in range(8)
    ]
    res = run_bass_kernel_spmd(
        nc, in_maps, core_ids=list(range(8)), trace=_trace,
        trace_cores=list(range(8)) if _trace else None,
    )
    parts = np.stack([res.results[c]["y"] for c in range(8)])  # [8, S, H]
    out = parts[0::2] + parts[1::2] + np.asarray(bo)[None, None, :]
    if _trace:
        kernel.last_results = res
    return out.astype(np.float32)



# revision 6
# speedup vs baseline: 1.1711x; 1.1711x over previous
"""Multi-head causal attention (B=4, S=2048, H=1024, 16 heads) on 8 TRN2 cores.

Sharding: batch (4) x head-group (2x8 heads) -> 8 cores. Each core computes,
for one batch and 8 heads: QKV projections, causal softmax attention, and its
partial output projection. Host sums the two head-group partials per batch and
adds the output bias (plus the algebraically-folded V bias: bv @ Wo.T).

v2 redesign (from baseline ~564us, PE/TensorMatrix-bound at 450us busy):
  - Softmax denominators come for free from the PV matmul via a ones column
    appended to each head's V tile ([ones|v] for odd heads so both heads of a
    pair land in partition-disjoint [0:65]/[63:128] ranges of their own PSUM
    bank). Removes the baseline's 320 one-row ones-matmuls (-68us PE).
  - K bias dropped (softmax shift-invariant), V bias folded into the host-side
    output bias (exact), Q bias kept on the device.
  - Causal column trimming: diagonal 128x512 score blocks only compute/exp/PV
    columns >= their offset (-20us PE).
  - Software pipelining: unit = (pair, head, 2-kt group); PV of unit i-1 is
    emitted after scores+exp of unit i, and projection / output-projection
    tiles are spread between attention units as PE filler, so the PE stays
    continuously busy (p-state ramp to 2.4 GHz) and never waits on ScalarE.
  - PSUM: 2x[128,1024] rotating slots (scores/proj/y) + 4x[128,512] (PV accs).
"""

import sys

sys.path.insert(0, "/opt/trn_rl_repo")

import math
from contextlib import ExitStack

import numpy as np
import ml_dtypes

import concourse.bass as bass
import concourse.mybir as mybir
from concourse import bacc
from concourse.tile import TileContext
from concourse.bass_utils import run_bass_kernel_spmd

BF16 = mybir.dt.bfloat16
F32 = mybir.dt.float32
AF = mybir.ActivationFunctionType
ALU = mybir.AluOpType

B, S, H = 4, 2048, 1024
NH, DH = 16, 64
O = 512          # per-core output dim of q/k/v projections (8 heads x 64)
NPAIR = 4        # head pairs per core
NSLAB = 4        # q slabs of 512
NST = 16         # s-tiles of 128
MASK_FILL = -8.0e5  # pre-scale (x0.125) additive mask for padded keys

_BUILT = {}


def _build(general_mask: bool):
    if general_mask in _BUILT:
        return _BUILT[general_mask]

    nc = bacc.Bacc("TRN2", target_bir_lowering=False, debug=False)

    xqT = nc.dram_tensor("xqT", [H, S], BF16, kind="ExternalInput")
    xkT = nc.dram_tensor("xkT", [H, S], BF16, kind="ExternalInput")
    xvT = nc.dram_tensor("xvT", [H, S], BF16, kind="ExternalInput")
    wqT = nc.dram_tensor("wqT", [H, O], BF16, kind="ExternalInput")
    wkT = nc.dram_tensor("wkT", [H, O], BF16, kind="ExternalInput")
    wvT = nc.dram_tensor("wvT", [H, O], BF16, kind="ExternalInput")
    woT = nc.dram_tensor("woT", [O, H], BF16, kind="ExternalInput")
    bqc = nc.dram_tensor("bqc", [128, 4], F32, kind="ExternalInput")
    tri = nc.dram_tensor("tri", [128, 128], BF16, kind="ExternalInput")
    mb = nc.dram_tensor("mb", [1, S], BF16, kind="ExternalInput")
    y = nc.dram_tensor("y", [S, H], F32, kind="ExternalOutput")
    yv = y.rearrange("s (j c) -> s j c", j=2)

    xq_r = xqT.rearrange("(po pi) s -> pi po s", pi=128)
    xk_r = xkT.rearrange("(po pi) s -> pi po s", pi=128)
    xv_r = xvT.rearrange("(po pi) s -> pi po s", pi=128)

    with TileContext(nc) as tc, ExitStack() as ctx:
        P = lambda name, bufs, **kw: ctx.enter_context(
            tc.tile_pool(name=name, bufs=bufs, **kw)
        )
        wp = P("wp", 1)
        xt = P("xt", 2)                    # x input chunks (bufs=2 per tag)
        qk = P("qk", 1)
        vp = P("vp", 1)
        pb = P("pb", 6)                    # probsT bf16
        ev = P("ev", 4)                    # recip rows + broadcast tiles
        ot = P("ot", 10)                   # o_sb bf16 (normalized outT)
        ys = P("ys", 2)                    # y sbuf staging
        p1 = P("p1", 2, space="PSUM")      # scores / proj / y psum [128,1024]
        p2 = P("p2", 4, space="PSUM")      # PV accumulators [128,512]

        # --- constants / weights ---
        wq_sb = wp.tile([128, 8, O], BF16, tag="wq", name="wq")
        wk_sb = wp.tile([128, 8, O], BF16, tag="wk", name="wk")
        wv_sb = wp.tile([128, 8, O], BF16, tag="wv", name="wv")
        wo_sb = wp.tile([128, 4, H], BF16, tag="wo", name="wo")
        bq_sb = wp.tile([128, 4], F32, tag="bq", name="bq")
        tri_sb = wp.tile([128, 128], BF16, tag="tri", name="tri")
        nc.sync.dma_start(wq_sb[:], wqT.rearrange("(po pi) o -> pi po o", pi=128))
        nc.sync.dma_start(wk_sb[:], wkT.rearrange("(po pi) o -> pi po o", pi=128))
        nc.sync.dma_start(wv_sb[:], wvT.rearrange("(po pi) o -> pi po o", pi=128))
        nc.sync.dma_start(wo_sb[:], woT.rearrange("(po pi) j -> pi po j", pi=128))
        nc.sync.dma_start(bq_sb[:], bqc[:, :])
        nc.sync.dma_start(tri_sb[:], tri[:, :])
        zeros_col = wp.tile([128, 1], F32, tag="zc", name="zc")
        nc.gpsimd.memset(zeros_col[:], 0.0)
        if general_mask:
            ones_row = wp.tile([1, 512], BF16, tag="onr", name="onr")
            nc.gpsimd.memset(ones_row[:], 1.0)
            mb_sb = wp.tile([1, S], BF16, tag="mb", name="mb")
            nc.sync.dma_start(mb_sb[:], mb[:, :])

        # v stored per (s-tile, head) as 65-col [v|ones] slots: each head's PV
        # lhsT is a contiguous [128, 65] slice; the ones column makes the PV
        # matmul emit the softmax denominator into out row 64 for free.
        v_sb = vp.tile([128, NST, 8, 65], BF16, tag="v", name="v")
        nc.gpsimd.memset(v_sb[:, :, :, 64:65], 1.0)

        # x chunks per 512-col range
        x_chunks = {"q": [None] * 4, "k": [None] * 4, "v": [None] * 4}

        def emit_x_chunk(which, r):
            src = {"q": xq_r, "k": xk_r, "v": xv_r}[which]
            t = xt.tile([128, 8, 512], BF16, tag=f"x{which}", name=f"x{which}")
            nc.sync.dma_start(t[:], src[:, :, 512 * r : 512 * r + 512])
            x_chunks[which][r] = t

        # persistent projections
        qT_sb = [qk.tile([128, S], BF16, tag=f"qT{p}", name=f"qT{p}") for p in range(NPAIR)]
        kT_sb = [qk.tile([128, S], BF16, tag=f"kT{p}", name=f"kT{p}") for p in range(NPAIR)]

        # --- projection tile emitters (each = one p1 grab, 16 matmuls) ---
        def emit_qk_tile(which, r, mpair):
            t = p1.tile([128, 2, 512], F32, tag="u", name="pj")
            xc = x_chunks[which][r]
            w_sb = wq_sb if which == "q" else wk_sb
            dst = qT_sb if which == "q" else kT_sb
            for half in range(2):
                m = 2 * mpair + half
                for ic in range(8):
                    nc.tensor.matmul(
                        t[:, half, :],
                        w_sb[:, ic, 128 * m : 128 * m + 128],
                        xc[:, ic, :],
                        start=(ic == 0),
                        stop=(ic == 7),
                    )
            for half in range(2):
                m = 2 * mpair + half
                if which == "q":
                    nc.vector.tensor_scalar_add(
                        dst[m][:, 512 * r : 512 * r + 512], t[:, half, :],
                        bq_sb[:, m : m + 1],
                    )
                else:
                    nc.vector.tensor_copy(
                        dst[m][:, 512 * r : 512 * r + 512], t[:, half, :]
                    )

        def emit_v_tile(r, half_idx):
            t = p1.tile([128, 2, 8, 64], F32, tag="u", name="pv")
            xc = x_chunks["v"][r]
            for half in range(2):
                st_local = 2 * half_idx + half
                for ic in range(8):
                    nc.tensor.matmul(
                        t[:, half, :, :],
                        xc[:, ic, 128 * st_local : 128 * st_local + 128],
                        wv_sb[:, ic, :],
                        start=(ic == 0),
                        stop=(ic == 7),
                    )
            for half in range(2):
                st = 4 * r + 2 * half_idx + half
                nc.vector.tensor_copy(v_sb[:, st, :, 0:64], t[:, half, :, :])

        # --- y projection tile emitter (one 128-row q stripe, full H) ---
        o_tiles = {}  # (slab, pair) -> o_sb

        def emit_y_tile(s, st):
            t = p1.tile([128, 2, 512], F32, tag="u", name="yt")
            for jsl in range(2):
                for p in range(NPAIR):
                    nc.tensor.matmul(
                        t[:, jsl, :],
                        o_tiles[(s, p)][:, 128 * st : 128 * st + 128],
                        wo_sb[:, p, 512 * jsl : 512 * jsl + 512],
                        start=(p == 0),
                        stop=(p == NPAIR - 1),
                    )
            ysb = ys.tile([128, 2, 512], F32, tag="ys", name="ysb")
            nc.vector.tensor_copy(ysb[:], t[:])
            srow = 512 * s + 128 * st
            nc.sync.dma_start(yv[srow : srow + 128, :, :], ysb[:])

        # --- attention emitters ---
        # unit = (slab, pair, head, g); group g covers k-tiles (2g, 2g+1)
        ot_ps = {}  # (pair, head) -> current PV psum tile

        def emit_scores(s, p, h, g):
            q0 = 512 * s
            r0 = 64 * h
            sc = p1.tile([128, 2, 512], F32, tag="u", name="sc")
            pb_t = pb.tile([128, 2, 512], BF16, tag="pb", name="pb")
            offs = []
            for j, kt in enumerate((2 * g, 2 * g + 1)):
                o = 128 * (kt - 4 * s) if kt >= 4 * s else 0
                offs.append(o)
                nc.tensor.matmul(
                    sc[:, j, o:512],
                    kT_sb[p][r0 : r0 + 64, 128 * kt : 128 * kt + 128],
                    qT_sb[p][r0 : r0 + 64, q0 + o : q0 + 512],
                    start=True,
                    stop=not general_mask,
                )
                if general_mask:
                    nc.tensor.matmul(
                        sc[:, j, o:512],
                        mb_sb[0:1, 128 * kt : 128 * kt + 128],
                        ones_row[0:1, 0 : 512 - o],
                        start=False,
                        stop=True,
                    )
            if offs[0] == 0 and offs[1] == 0:
                nc.scalar.activation(
                    pb_t[:, :, :], sc[:, :, :], AF.Exp,
                    bias=zeros_col[:, 0:1], scale=0.125,
                )
            else:
                for j, o in enumerate(offs):
                    nc.scalar.activation(
                        pb_t[:, j, o:512], sc[:, j, o:512], AF.Exp,
                        bias=zeros_col[:, 0:1], scale=0.125,
                    )
            for j, kt in enumerate((2 * g, 2 * g + 1)):
                if kt >= 4 * s:  # diagonal block: mask the 128-wide triangle band
                    o = offs[j]
                    nc.vector.tensor_tensor(
                        pb_t[:, j, o : o + 128],
                        pb_t[:, j, o : o + 128],
                        tri_sb[:, 0:128],
                        ALU.mult,
                    )
            return pb_t

        def emit_pv(s, p, h, g, pb_t):
            n_kt = 4 * (s + 1)
            if g == 0:
                ot_ps[(p, h)] = p2.tile([128, 512], F32, tag="ot", name="ot")
            acc = ot_ps[(p, h)]
            for j, kt in enumerate((2 * g, 2 * g + 1)):
                o = 128 * (kt - 4 * s) if kt >= 4 * s else 0
                nc.tensor.matmul(
                    acc[0:65, o:512],
                    v_sb[:, kt, 2 * p + h, 0:65],
                    pb_t[:, j, o:512],
                    start=(kt == 0),
                    stop=(kt == n_kt - 1),
                )

        def emit_norm(s, p):
            accA = ot_ps[(p, 0)]
            accB = ot_ps[(p, 1)]
            lrA = ev.tile([1, 512], F32, tag="lr", name="lrA")
            lrB = ev.tile([1, 512], F32, tag="lr", name="lrB")
            nc.vector.reciprocal(lrA[:], accA[64:65, :])
            nc.vector.reciprocal(lrB[:], accB[64:65, :])
            rxA = ev.tile([64, 512], F32, tag="rx", name="rxA")
            rxB = ev.tile([64, 512], F32, tag="rx", name="rxB")
            nc.gpsimd.partition_broadcast(rxA[:], lrA[:])
            nc.gpsimd.partition_broadcast(rxB[:], lrB[:])
            o_sb = ot.tile([128, 512], BF16, tag="ot", name="osb")
            nc.vector.tensor_tensor(
                o_sb[0:64, :], accA[0:64, :], rxA[:, :], ALU.mult
            )
            nc.vector.tensor_tensor(
                o_sb[64:128, :], accB[0:64, :], rxB[:, :], ALU.mult
            )
            o_tiles[(s, p)] = o_sb

        # --- emission schedule ---
        for r in range(4):
            for which in ("q", "k", "v"):
                emit_x_chunk(which, r)

        for mpair in range(2):
            emit_qk_tile("q", 0, mpair)
        for mpair in range(2):
            emit_qk_tile("k", 0, mpair)
        for half_idx in range(2):
            emit_v_tile(0, half_idx)

        pending = None  # (s, p, h, g, pb_t) awaiting PV
        for s in range(NSLAB):
            fillers = []
            if s < 3:
                r = s + 1
                fillers += [
                    lambda r=r: emit_qk_tile("q", r, 0),
                    lambda r=r: emit_qk_tile("q", r, 1),
                    lambda r=r: emit_qk_tile("k", r, 0),
                    lambda r=r: emit_qk_tile("k", r, 1),
                    lambda r=r: emit_v_tile(r, 0),
                    lambda r=r: emit_v_tile(r, 1),
                ]
            if s >= 1:
                for st in range(4):
                    fillers.append(lambda s=s, st=st: emit_y_tile(s - 1, st))

            units = [
                (p, h, g)
                for p in range(NPAIR)
                for g in range(2 * (s + 1))
                for h in range(2)
            ]
            every = max(1, -(-len(units) // max(1, len(fillers))))
            fi = 0
            for i, (p, h, g) in enumerate(units):
                pb_t = emit_scores(s, p, h, g)
                if fi < len(fillers) and i % every == every - 1:
                    fillers[fi]()
                    fi += 1
                if pending is not None:
                    ps_, pp_, ph_, pg_, ppb = pending
                    emit_pv(ps_, pp_, ph_, pg_, ppb)
                    if ph_ == 1 and pg_ == 2 * (ps_ + 1) - 1:
                        emit_norm(ps_, pp_)
                pending = (s, p, h, g, pb_t)
            while fi < len(fillers):
                fillers[fi]()
                fi += 1

        ps_, pp_, ph_, pg_, ppb = pending
        emit_pv(ps_, pp_, ph_, pg_, ppb)
        emit_norm(ps_, pp_)
        for st in range(4):
            emit_y_tile(3, st)

    nc.compile()
    _BUILT[general_mask] = nc
    return nc


def _prep_core(query, key, value, mask, Wq, Wk, Wv, Wo, bq, core):
    b, hg = core // 2, core % 2
    o_sl = slice(hg * O, hg * O + O)
    bf = ml_dtypes.bfloat16

    tri = np.zeros((128, 128), dtype=np.float32)
    d = np.arange(128)[None, :]
    kk = np.arange(128)[:, None]
    tri[d >= kk] = 1.0

    mrow = np.where(mask[b] > 0, 0.0, MASK_FILL).astype(np.float32)

    return {
        "xqT": np.ascontiguousarray(query[b].T).astype(bf),
        "xkT": np.ascontiguousarray(key[b].T).astype(bf),
        "xvT": np.ascontiguousarray(value[b].T).astype(bf),
        "wqT": np.ascontiguousarray(Wq[o_sl].T).astype(bf),
        "wkT": np.ascontiguousarray(Wk[o_sl].T).astype(bf),
        "wvT": np.ascontiguousarray(Wv[o_sl].T).astype(bf),
        "woT": np.ascontiguousarray(Wo[:, o_sl].T).astype(bf),
        "bqc": np.ascontiguousarray(bq[o_sl].reshape(4, 128).T).astype(np.float32),
        "tri": tri.astype(bf),
        "mb": mrow.reshape(1, S).astype(bf),
    }


def kernel(query, key, value, mask, Wq, bq, Wk, bk, Wv, bv, Wo, bo, _trace=False):
    general_mask = bool(np.any(np.asarray(mask) <= 0))
    nc = _build(general_mask)
    in_maps = [
        _prep_core(query, key, value, mask, Wq, Wk, Wv, Wo, bq, c)
        for c in range(8)
    ]
    res = run_bass_kernel_spmd(
        nc, in_maps, core_ids=list(range(8)), trace=_trace,
        trace_cores=list(range(8)) if _trace else None,
    )
    parts = np.stack([res.results[c]["y"] for c in range(8)])  # [8, S, H]
    # V bias folded through the output projection (exact: sum_k probs = 1
    # after normalization); K bias dropped (softmax shift-invariant).
    bias = np.asarray(bo) + np.asarray(bv) @ np.asarray(Wo).T
    out = parts[0::2] + parts[1::2] + bias[None, None, :]
    if _trace:
        kernel.last_results = res
    return out.astype(np.float32)


# revision 54
# speedup vs baseline: 1.6255x; 1.3879x over previous
"""Multi-head causal attention (B=4, S=2048, H=1024, 16 heads) on 8 TRN2 cores.

Sharding: batch (4) x head-group (2x8 heads) -> 8 cores. Each core computes,
for one batch and 8 heads: QKV projections, causal softmax attention, and its
partial output projection. Host sums the two head-group partials per batch and
adds the output bias (plus the algebraically-folded V bias: bv @ Wo.T).

v2 redesign (from baseline ~564us, PE/TensorMatrix-bound at 450us busy):
  - Softmax denominators come for free from the PV matmul via a ones column
    appended to each head's V tile ([ones|v] for odd heads so both heads of a
    pair land in partition-disjoint [0:65]/[63:128] ranges of their own PSUM
    bank). Removes the baseline's 320 one-row ones-matmuls (-68us PE).
  - K bias dropped (softmax shift-invariant), V bias folded into the host-side
    output bias (exact), Q bias kept on the device.
  - Causal column trimming: diagonal 128x512 score blocks only compute/exp/PV
    columns >= their offset (-20us PE).
  - Software pipelining: unit = (pair, head, 2-kt group); PV of unit i-1 is
    emitted after scores+exp of unit i, and projection / output-projection
    tiles are spread between attention units as PE filler, so the PE stays
    continuously busy (p-state ramp to 2.4 GHz) and never waits on ScalarE.
  - PSUM: 2x[128,1024] rotating slots (scores/proj/y) + 4x[128,512] (PV accs).
"""

import sys

sys.path.insert(0, "/opt/trn_rl_repo")

import math
from contextlib import ExitStack

import numpy as np
import ml_dtypes

import concourse.bass as bass
import concourse.mybir as mybir
from concourse import bacc
from concourse.hw_specs import get_activation_tables
from concourse.tile import TileContext
from concourse.bass_utils import run_bass_kernel_spmd

BF16 = mybir.dt.bfloat16
F32 = mybir.dt.float32
AF = mybir.ActivationFunctionType
ALU = mybir.AluOpType

B, S, H = 4, 2048, 1024
NH, DH = 16, 64
O = 512          # per-core output dim of q/k/v projections (8 heads x 64)
NPAIR = 4        # head pairs per core
NSLAB = 4        # q slabs of 512
NST = 16         # s-tiles of 128
MASK_FILL = -8.0e5  # pre-scale (x0.125) additive mask for padded keys

_BUILT = {}


def _build(general_mask: bool):
    if general_mask in _BUILT:
        return _BUILT[general_mask]

    nc = bacc.Bacc("TRN2", target_bir_lowering=False, debug=False)

    xqT = nc.dram_tensor("xqT", [H, S], BF16, kind="ExternalInput")
    xkT = nc.dram_tensor("xkT", [H, S], BF16, kind="ExternalInput")
    xvT = nc.dram_tensor("xvT", [H, S], BF16, kind="ExternalInput")
    wqT = nc.dram_tensor("wqT", [H, O], BF16, kind="ExternalInput")
    wkT = nc.dram_tensor("wkT", [H, O], BF16, kind="ExternalInput")
    wvT = nc.dram_tensor("wvT", [H, O], BF16, kind="ExternalInput")
    woT = nc.dram_tensor("woT", [O, H], BF16, kind="ExternalInput")
    bqc = nc.dram_tensor("bqc", [128, 4], F32, kind="ExternalInput")
    tri = nc.dram_tensor("tri", [128, 128], BF16, kind="ExternalInput")
    mb = nc.dram_tensor("mb", [1, S], BF16, kind="ExternalInput")
    y = nc.dram_tensor("y", [S, H], BF16, kind="ExternalOutput")
    yv = y.rearrange("s (j c) -> s j c", j=2)

    xq_r = xqT.rearrange("(po pi) s -> pi po s", pi=128)
    xk_r = xkT.rearrange("(po pi) s -> pi po s", pi=128)
    xv_r = xvT.rearrange("(po pi) s -> pi po s", pi=128)

    with TileContext(nc) as tc, ExitStack() as ctx:
        P = lambda name, bufs, **kw: ctx.enter_context(
            tc.tile_pool(name=name, bufs=bufs, **kw)
        )
        wp = P("wp", 1)
        xt = P("xt", 2)                    # x input chunks (bufs=2 per tag)
        qk = P("qk", 1)
        vp = P("vp", 1)
        pb = P("pb", 6)                    # probsT bf16
        ev = P("ev", 4)                    # recip rows + broadcast tiles
        ot = P("ot", 10)                   # o_sb bf16 (normalized outT)
        ys = P("ys", 2)                    # y sbuf staging
        p1 = P("p1", 2, space="PSUM")      # scores / proj / y psum [128,1024]
        p2 = P("p2", 4, space="PSUM")      # PV accumulators [128,512]

        # --- constants / weights ---
        wq_sb = wp.tile([128, 8, O], BF16, tag="wq", name="wq")
        wk_sb = wp.tile([128, 8, O], BF16, tag="wk", name="wk")
        wv_sb = wp.tile([128, 8, O], BF16, tag="wv", name="wv")
        wo_sb = wp.tile([128, 4, H], BF16, tag="wo", name="wo")
        bq_sb = wp.tile([128, 4], F32, tag="bq", name="bq")
        tri_sb = wp.tile([128, 128], BF16, tag="tri", name="tri")
        zeros_col = wp.tile([128, 1], F32, tag="zc", name="zc")
        nc.gpsimd.memset(zeros_col[:], 0.0)
        ones_t = wp.tile([33, 64], BF16, tag="onest", name="onest")
        nc.gpsimd.memset(ones_t[:], 1.0)

        # Pre-load the one activation table containing BOTH Exp and Ln, so the
        # insert_act_table_loads fixpoint never has to thrash between the
        # exp-only and ln-only sets (measured 33 x 1.28us of reloads without
        # this).
        af_sets = list(get_activation_tables(nc.m.arch).items())
        joint = next(
            i for i, (_, fns) in enumerate(af_sets)
            if AF.Exp in fns and AF.Ln in fns
        )
        nc.scalar.add_instruction(
            mybir.InstLoadActFuncSet(
                name=nc.get_next_instruction_name(),
                ins=[],
                outs=[],
                act_func_set_id=joint,
            )
        )
        if general_mask:
            ones_row = wp.tile([1, 512], BF16, tag="onr", name="onr")
            nc.gpsimd.memset(ones_row[:], 1.0)
            mb_sb = wp.tile([1, S], BF16, tag="mb", name="mb")
            nc.sync.dma_start(mb_sb[:], mb[:, :])

        # v stored per (s-tile, head) as 65-col [v|ones] slots: each head's PV
        # lhsT is a contiguous [128, 65] slice; the ones column makes the PV
        # matmul emit the softmax denominator into out row 64 for free.
        v_sb = vp.tile([128, NST, 8, 65], BF16, tag="v", name="v")
        nc.gpsimd.memset(v_sb[:, :, :, 64:65], 1.0)

        # x chunks per 512-col range
        x_chunks = {"q": [None] * 4, "k": [None] * 4, "v": [None] * 4}

        def emit_x_chunk(which, r, split=False):
            src = {"q": xq_r, "k": xk_r, "v": xv_r}[which]
            t = xt.tile([128, 8, 512], BF16, tag=f"x{which}", name=f"x{which}")
            if split:  # startup path: halves land (and unblock) sooner
                nc.sync.dma_start(t[:, 0:4, :], src[:, 0:4, 512 * r : 512 * r + 512])
                nc.sync.dma_start(t[:, 4:8, :], src[:, 4:8, 512 * r : 512 * r + 512])
            else:
                nc.sync.dma_start(t[:], src[:, :, 512 * r : 512 * r + 512])
            x_chunks[which][r] = t

        # persistent projections
        qT_sb = [qk.tile([128, S], BF16, tag=f"qT{p}", name=f"qT{p}") for p in range(NPAIR)]
        kT_sb = [qk.tile([128, S], BF16, tag=f"kT{p}", name=f"kT{p}") for p in range(NPAIR)]

        # --- projection tile emitters (each = one p1 grab, 16 matmuls) ---
        def emit_qk_tile(which, r, mpair):
            t = p1.tile([128, 2, 512], F32, tag="u", name="pj")
            xc = x_chunks[which][r]
            w_sb = wq_sb if which == "q" else wk_sb
            dst = qT_sb if which == "q" else kT_sb
            for half in range(2):
                m = 2 * mpair + half
                for ic in range(8):
                    nc.tensor.matmul(
                        t[:, half, :],
                        w_sb[:, ic, 128 * m : 128 * m + 128],
                        xc[:, ic, :],
                        start=(ic == 0),
                        stop=(ic == 7),
                    )
            for half in range(2):
                m = 2 * mpair + half
                if which == "q":
                    nc.vector.tensor_scalar_add(
                        dst[m][:, 512 * r : 512 * r + 512], t[:, half, :],
                        bq_sb[:, m : m + 1],
                    )
                else:
                    nc.vector.tensor_copy(
                        dst[m][:, 512 * r : 512 * r + 512], t[:, half, :]
                    )

        def emit_v_tile(r, half_idx):
            t = p1.tile([128, 2, 8, 64], F32, tag="u", name="pv")
            xc = x_chunks["v"][r]
            for half in range(2):
                st_local = 2 * half_idx + half
                for ic in range(8):
                    nc.tensor.matmul(
                        t[:, half, :, :],
                        xc[:, ic, 128 * st_local : 128 * st_local + 128],
                        wv_sb[:, ic, :],
                        start=(ic == 0),
                        stop=(ic == 7),
                    )
            for half in range(2):
                st = 4 * r + 2 * half_idx + half
                nc.vector.tensor_copy(v_sb[:, st, :, 0:64], t[:, half, :, :])

        # --- y projection tile emitter (one 128-row q stripe, full H) ---
        o_tiles = {}  # (slab, pair) -> o_sb

        def emit_y_tile(s, st):
            t = p1.tile([128, 2, 512], F32, tag="u", name="yt")
            for jsl in range(2):
                for p in range(NPAIR):
                    nc.tensor.matmul(
                        t[:, jsl, :],
                        o_tiles[(s, p)][:, 128 * st : 128 * st + 128],
                        wo_sb[:, p, 512 * jsl : 512 * jsl + 512],
                        start=(p == 0),
                        stop=(p == NPAIR - 1),
                    )
            ysb = ys.tile([128, 2, 512], BF16, tag="ys", name="ysb")
            nc.vector.tensor_copy(ysb[:], t[:])
            srow = 512 * s + 128 * st
            nc.sync.dma_start(yv[srow : srow + 128, :, :], ysb[:])

        # --- attention emitters ---
        # unit = (slab, pair, head, g); group g covers k-tiles (2g, 2g+1)
        ot_ps = {}  # (pair, head) -> current PV psum tile

        def emit_scores(s, p, h, g):
            q0 = 512 * s
            r0 = 64 * h
            sc = p1.tile([128, 2, 512], F32, tag="u", name="sc")
            pb_t = pb.tile([128, 2, 512], BF16, tag="pb", name="pb")
            offs = []
            for j, kt in enumerate((2 * g, 2 * g + 1)):
                o = 128 * (kt - 4 * s) if kt >= 4 * s else 0
                offs.append(o)
                nc.tensor.matmul(
                    sc[:, j, o:512],
                    kT_sb[p][r0 : r0 + 64, 128 * kt : 128 * kt + 128],
                    qT_sb[p][r0 : r0 + 64, q0 + o : q0 + 512],
                    start=True,
                    stop=not general_mask,
                )
                if general_mask:
                    nc.tensor.matmul(
                        sc[:, j, o:512],
                        mb_sb[0:1, 128 * kt : 128 * kt + 128],
                        ones_row[0:1, 0 : 512 - o],
                        start=False,
                        stop=True,
                    )
            if offs[0] == 0 and offs[1] == 0:
                nc.scalar.activation(
                    pb_t[:, :, :], sc[:, :, :], AF.Exp,
                    bias=zeros_col[:, 0:1], scale=0.125,
                )
            else:
                for j, o in enumerate(offs):
                    nc.scalar.activation(
                        pb_t[:, j, o:512], sc[:, j, o:512], AF.Exp,
                        bias=zeros_col[:, 0:1], scale=0.125,
                    )
            for j, kt in enumerate((2 * g, 2 * g + 1)):
                if kt >= 4 * s:  # diagonal block: mask the 128-wide triangle band
                    o = offs[j]
                    nc.vector.tensor_tensor(
                        pb_t[:, j, o : o + 128],
                        pb_t[:, j, o : o + 128],
                        tri_sb[:, 0:128],
                        ALU.mult,
                    )
            return pb_t

        def emit_pv(s, p, h, g, pb_t):
            n_kt = 4 * (s + 1)
            if g == 0:
                ot_ps[(p, h)] = p2.tile([128, 512], F32, tag="ot", name="ot")
            acc = ot_ps[(p, h)]
            for j, kt in enumerate((2 * g, 2 * g + 1)):
                o = 128 * (kt - 4 * s) if kt >= 4 * s else 0
                nc.tensor.matmul(
                    acc[0:65, o:512],
                    v_sb[:, kt, 2 * p + h, 0:65],
                    pb_t[:, j, o:512],
                    start=(kt == 0),
                    stop=(kt == n_kt - 1),
                )

        # Normalization, fully per-pair (spread evenly, no DVE lumps):
        # evacuate both PV banks to SBUF (frees PSUM early), gather the two
        # denominator rows into a [33,512] tile at legal bases {0,32}, compute
        # 1/L = exp(-ln L) on the Scalar engine (ln and exp share one
        # activation table, so no table thrash; ~0.65us per op vs a 4us DVE
        # reciprocal), broadcast each row with a K=1 ones matmul on the PE
        # into p2 PSUM, and finish with two DVE multiplies.
        norm_q = []  # deferred finish thunks: [countdown, thunk]
        o_parts = {}  # (s, p) -> (osfA, osfB) awaiting finish_norm

        def emit_norm(s, p):
            accA = ot_ps[(p, 0)]
            accB = ot_ps[(p, 1)]
            osfA = ev.tile([64, 512], F32, tag="osf", name="osfA", bufs=10)
            osfB = ev.tile([64, 512], F32, tag="osf", name="osfB", bufs=10)
            nc.vector.tensor_copy(osfA[:], accA[0:64, :])
            nc.vector.tensor_copy(osfB[:], accB[0:64, :])
            lt = ev.tile([33, 512], F32, tag="Lt", name="lt", bufs=4)
            nc.gpsimd.memset(lt[:], 1.0)
            nc.vector.tensor_copy(lt[0:1, :], accA[64:65, :])
            nc.vector.tensor_copy(lt[32:33, :], accB[64:65, :])
            lnt = ev.tile([33, 512], F32, tag="Lt", name="lnt", bufs=4)
            nc.scalar.activation(
                lnt[:], lt[:], AF.Ln, bias=zeros_col[0:33, 0:1], scale=1.0
            )
            rb = ev.tile([33, 512], BF16, tag="rb", name="rb", bufs=4)
            nc.scalar.activation(
                rb[:], lnt[:], AF.Exp, bias=zeros_col[0:33, 0:1], scale=-1.0
            )
            o_parts[(s, p)] = (osfA, osfB)
            # The PE broadcast + DVE multiply wait on the ACT ln/exp chain, so
            # defer them a few units: by then the reciprocal row is ready and
            # the in-order PE queue never stalls on it.
            norm_q.append([3, lambda: finish_norm(s, p, rb)])

        def finish_norm(s, p, rb):
            osfA, osfB = o_parts.pop((s, p))
            rxA = p2.tile([64, 512], F32, tag="ot", name="rxA")
            nc.tensor.matmul(
                rxA[:], ones_t[0:1, :], rb[0:1, :], start=True, stop=True
            )
            rxB = p2.tile([64, 512], F32, tag="ot", name="rxB")
            nc.tensor.matmul(
                rxB[:], ones_t[32:33, :], rb[32:33, :], start=True, stop=True
            )
            o_sb = ot.tile([128, 512], BF16, tag="ot", name="osb")
            nc.vector.tensor_tensor(o_sb[0:64, :], osfA[:, :], rxA[:, :], ALU.mult)
            nc.vector.tensor_tensor(o_sb[64:128, :], osfB[:, :], rxB[:, :], ALU.mult)
            o_tiles[(s, p)] = o_sb

        # --- emission schedule ---
        # DMA order: interleave weights with the range-0 x chunks so the first
        # projection tile's inputs arrive as early as possible.
        wq_r = wqT.rearrange("(po pi) o -> pi po o", pi=128)
        nc.sync.dma_start(wq_sb[:, :, 0:256], wq_r[:, :, 0:256])
        emit_x_chunk("q", 0, split=True)
        nc.sync.dma_start(wq_sb[:, :, 256:512], wq_r[:, :, 256:512])
        nc.sync.dma_start(bq_sb[:], bqc[:, :])
        nc.sync.dma_start(wk_sb[:], wkT.rearrange("(po pi) o -> pi po o", pi=128))
        emit_x_chunk("k", 0, split=True)
        nc.sync.dma_start(wv_sb[:], wvT.rearrange("(po pi) o -> pi po o", pi=128))
        emit_x_chunk("v", 0)
        nc.sync.dma_start(tri_sb[:], tri[:, :])
        nc.sync.dma_start(wo_sb[:], woT.rearrange("(po pi) j -> pi po j", pi=128))
        for r in range(1, 4):
            for which in ("q", "k", "v"):
                emit_x_chunk(which, r)

        for mpair in range(2):
            emit_qk_tile("q", 0, mpair)
        for mpair in range(2):
            emit_qk_tile("k", 0, mpair)
        for half_idx in range(2):
            emit_v_tile(0, half_idx)

        # Filler distribution: k(s)/v(s) tiles land at the START of slab s
        # (just-in-time for k-tiles 4s..4s+3, first consumed at unit 4s) so
        # each slab opens with a long dense PE burst; q(s+1) spreads across
        # slab s; y tiles backfill the ACT-paced late slabs.
        front_fillers = {
            1: [lambda: emit_qk_tile("k", 1, 0), lambda: emit_qk_tile("k", 1, 1),
                lambda: emit_v_tile(1, 0), lambda: emit_v_tile(1, 1)],
            2: [lambda: emit_qk_tile("k", 2, 0), lambda: emit_qk_tile("k", 2, 1),
                lambda: emit_v_tile(2, 0), lambda: emit_v_tile(2, 1)],
            3: [lambda: emit_qk_tile("k", 3, 0), lambda: emit_qk_tile("k", 3, 1),
                lambda: emit_v_tile(3, 0), lambda: emit_v_tile(3, 1)],
        }
        spread_fillers = {
            0: [lambda: emit_qk_tile("q", 1, 0), lambda: emit_qk_tile("q", 1, 1)],
            1: [lambda: emit_qk_tile("q", 2, 0), lambda: emit_qk_tile("q", 2, 1)],
            2: [lambda: emit_qk_tile("q", 3, 0), lambda: emit_qk_tile("q", 3, 1)]
               + [lambda st=st: emit_y_tile(0, st) for st in range(4)],
            3: [lambda st=st: emit_y_tile(1, st) for st in range(4)]
               + [lambda st=st: emit_y_tile(2, st) for st in range(4)],
        }

        pending = []  # up to 2 units (s, p, h, g, pb_t) awaiting PV (lag-2)
        for s in range(NSLAB):
            front = list(front_fillers.get(s, []))
            fillers = list(spread_fillers.get(s, []))
            units = [
                (p, h, g)
                for p in range(NPAIR)
                for g in range(2 * (s + 1))
                for h in range(2)
            ]
            every = max(1, -(-len(units) // max(1, len(fillers))))
            fi = 0
            for i, (p, h, g) in enumerate(units):
                pb_t = emit_scores(s, p, h, g)
                if i < len(front):
                    front[i]()
                elif fi < len(fillers) and i % every == every - 1:
                    fillers[fi]()
                    fi += 1
                if len(pending) >= 2:
                    ps_, pp_, ph_, pg_, ppb = pending.pop(0)
                    emit_pv(ps_, pp_, ph_, pg_, ppb)
                    if ph_ == 1 and pg_ == 2 * (ps_ + 1) - 1:
                        emit_norm(ps_, pp_)
                for item in norm_q:
                    item[0] -= 1
                while norm_q and norm_q[0][0] <= 0:
                    norm_q.pop(0)[1]()
                pending.append((s, p, h, g, pb_t))
            while fi < len(fillers):
                fillers[fi]()
                fi += 1

        while pending:
            ps_, pp_, ph_, pg_, ppb = pending.pop(0)
            emit_pv(ps_, pp_, ph_, pg_, ppb)
            if ph_ == 1 and pg_ == 2 * (ps_ + 1) - 1:
                emit_norm(ps_, pp_)
        while norm_q:
            norm_q.pop(0)[1]()
        for st in range(4):
            emit_y_tile(3, st)

    nc.compile()
    _BUILT[general_mask] = nc
    return nc


def _prep_core(query, key, value, mask, Wq, Wk, Wv, Wo, bq, core):
    b, hg = core // 2, core % 2
    o_sl = slice(hg * O, hg * O + O)
    bf = ml_dtypes.bfloat16

    d = np.arange(128)[None, :]
    kk = np.arange(128)[:, None]
    tri = (d >= kk).astype(np.float32)

    mrow = np.where(mask[b] > 0, 0.0, MASK_FILL).astype(np.float32)

    return {
        "xqT": np.ascontiguousarray(query[b].T).astype(bf),
        "xkT": np.ascontiguousarray(key[b].T).astype(bf),
        "xvT": np.ascontiguousarray(value[b].T).astype(bf),
        "wqT": np.ascontiguousarray(Wq[o_sl].T).astype(bf),
        "wkT": np.ascontiguousarray(Wk[o_sl].T).astype(bf),
        "wvT": np.ascontiguousarray(Wv[o_sl].T).astype(bf),
        "woT": np.ascontiguousarray(Wo[:, o_sl].T).astype(bf),
        "bqc": np.ascontiguousarray(bq[o_sl].reshape(4, 128).T).astype(np.float32),
        "tri": tri.astype(bf),
        "mb": mrow.reshape(1, S).astype(bf),
    }


def kernel(query, key, value, mask, Wq, bq, Wk, bk, Wv, bv, Wo, bo, _trace=False):
    general_mask = bool(np.any(np.asarray(mask) <= 0))
    nc = _build(general_mask)
    in_maps = [
        _prep_core(query, key, value, mask, Wq, Wk, Wv, Wo, bq, c)
        for c in range(8)
    ]
    res = run_bass_kernel_spmd(
        nc, in_maps, core_ids=list(range(8)), trace=_trace,
        trace_cores=list(range(8)) if _trace else None,
    )
    parts = np.stack(
        [np.asarray(res.results[c]["y"], dtype=np.float32) for c in range(8)]
    )  # [8, S, H]
    # V bias folded through the output projection (exact: sum_k probs = 1
    # after normalization); K bias dropped (softmax shift-invariant).
    bias = np.asarray(bo) + np.asarray(bv) @ np.asarray(Wo).T
    out = parts[0::2] + parts[1::2] + bias[None, None, :]
    if _trace:
        kernel.last_results = res
    return out.astype(np.float32)


# revision 56
# speedup vs baseline: 1.6280x; 1.0015x over previous
"""Multi-head causal attention (B=4, S=2048, H=1024, 16 heads) on 8 TRN2 cores.

Sharding: batch (4) x head-group (2x8 heads) -> 8 cores. Each core computes,
for one batch and 8 heads: QKV projections, causal softmax attention, and its
partial output projection. Host sums the two head-group partials per batch and
adds the output bias (plus the algebraically-folded V bias: bv @ Wo.T).

v2 redesign (from baseline ~564us, PE/TensorMatrix-bound at 450us busy):
  - Softmax denominators come for free from the PV matmul via a ones column
    appended to each head's V tile ([ones|v] for odd heads so both heads of a
    pair land in partition-disjoint [0:65]/[63:128] ranges of their own PSUM
    bank). Removes the baseline's 320 one-row ones-matmuls (-68us PE).
  - K bias dropped (softmax shift-invariant), V bias folded into the host-side
    output bias (exact), Q bias kept on the device.
  - Causal column trimming: diagonal 128x512 score blocks only compute/exp/PV
    columns >= their offset (-20us PE).
  - Software pipelining: unit = (pair, head, 2-kt group); PV of unit i-1 is
    emitted after scores+exp of unit i, and projection / output-projection
    tiles are spread between attention units as PE filler, so the PE stays
    continuously busy (p-state ramp to 2.4 GHz) and never waits on ScalarE.
  - PSUM: 2x[128,1024] rotating slots (scores/proj/y) + 4x[128,512] (PV accs).
"""

import sys

sys.path.insert(0, "/opt/trn_rl_repo")

import math
from contextlib import ExitStack

import numpy as np
import ml_dtypes

import concourse.bass as bass
import concourse.mybir as mybir
from concourse import bacc
from concourse.hw_specs import get_activation_tables
from concourse.tile import TileContext
from concourse.bass_utils import run_bass_kernel_spmd

BF16 = mybir.dt.bfloat16
F32 = mybir.dt.float32
AF = mybir.ActivationFunctionType
ALU = mybir.AluOpType

B, S, H = 4, 2048, 1024
NH, DH = 16, 64
O = 512          # per-core output dim of q/k/v projections (8 heads x 64)
NPAIR = 4        # head pairs per core
NSLAB = 4        # q slabs of 512
NST = 16         # s-tiles of 128
MASK_FILL = -8.0e5  # pre-scale (x0.125) additive mask for padded keys

_BUILT = {}


def _build(general_mask: bool):
    if general_mask in _BUILT:
        return _BUILT[general_mask]

    nc = bacc.Bacc("TRN2", target_bir_lowering=False, debug=False)

    xqT = nc.dram_tensor("xqT", [H, S], BF16, kind="ExternalInput")
    xkT = nc.dram_tensor("xkT", [H, S], BF16, kind="ExternalInput")
    xvT = nc.dram_tensor("xvT", [H, S], BF16, kind="ExternalInput")
    wqT = nc.dram_tensor("wqT", [H, O], BF16, kind="ExternalInput")
    wkT = nc.dram_tensor("wkT", [H, O], BF16, kind="ExternalInput")
    wvT = nc.dram_tensor("wvT", [H, O], BF16, kind="ExternalInput")
    woT = nc.dram_tensor("woT", [O, H], BF16, kind="ExternalInput")
    bqc = nc.dram_tensor("bqc", [128, 4], F32, kind="ExternalInput")
    tri = nc.dram_tensor("tri", [128, 128], BF16, kind="ExternalInput")
    mb = nc.dram_tensor("mb", [1, S], BF16, kind="ExternalInput")
    y = nc.dram_tensor("y", [S, H], BF16, kind="ExternalOutput")
    yv = y.rearrange("s (j c) -> s j c", j=2)

    xq_r = xqT.rearrange("(po pi) s -> pi po s", pi=128)
    xk_r = xkT.rearrange("(po pi) s -> pi po s", pi=128)
    xv_r = xvT.rearrange("(po pi) s -> pi po s", pi=128)

    with TileContext(nc) as tc, ExitStack() as ctx:
        P = lambda name, bufs, **kw: ctx.enter_context(
            tc.tile_pool(name=name, bufs=bufs, **kw)
        )
        wp = P("wp", 1)
        xt = P("xt", 2)                    # x input chunks (bufs=2 per tag)
        qk = P("qk", 1)
        vp = P("vp", 1)
        pb = P("pb", 6)                    # probsT bf16
        ev = P("ev", 4)                    # recip rows + broadcast tiles
        ot = P("ot", 10)                   # o_sb bf16 (normalized outT)
        ys = P("ys", 2)                    # y sbuf staging
        p1 = P("p1", 2, space="PSUM")      # scores / proj / y psum [128,1024]
        p2 = P("p2", 4, space="PSUM")      # PV accumulators [128,512]

        # --- constants / weights ---
        wq_sb = wp.tile([128, 8, O], BF16, tag="wq", name="wq")
        wk_sb = wp.tile([128, 8, O], BF16, tag="wk", name="wk")
        wv_sb = wp.tile([128, 8, O], BF16, tag="wv", name="wv")
        wo_sb = wp.tile([128, 4, H], BF16, tag="wo", name="wo")
        bq_sb = wp.tile([128, 4], F32, tag="bq", name="bq")
        tri_sb = wp.tile([128, 128], BF16, tag="tri", name="tri")
        zeros_col = wp.tile([128, 1], F32, tag="zc", name="zc")
        nc.gpsimd.memset(zeros_col[:], 0.0)
        ones_t = wp.tile([33, 64], BF16, tag="onest", name="onest")
        nc.gpsimd.memset(ones_t[:], 1.0)

        # Pre-load the one activation table containing BOTH Exp and Ln, so the
        # insert_act_table_loads fixpoint never has to thrash between the
        # exp-only and ln-only sets (measured 33 x 1.28us of reloads without
        # this).
        af_sets = list(get_activation_tables(nc.m.arch).items())
        joint = next(
            i for i, (_, fns) in enumerate(af_sets)
            if AF.Exp in fns and AF.Ln in fns
        )
        nc.scalar.add_instruction(
            mybir.InstLoadActFuncSet(
                name=nc.get_next_instruction_name(),
                ins=[],
                outs=[],
                act_func_set_id=joint,
            )
        )
        if general_mask:
            ones_row = wp.tile([1, 512], BF16, tag="onr", name="onr")
            nc.gpsimd.memset(ones_row[:], 1.0)
            mb_sb = wp.tile([1, S], BF16, tag="mb", name="mb")
            nc.sync.dma_start(mb_sb[:], mb[:, :])

        # v stored per (s-tile, head) as 65-col [v|ones] slots: each head's PV
        # lhsT is a contiguous [128, 65] slice; the ones column makes the PV
        # matmul emit the softmax denominator into out row 64 for free.
        v_sb = vp.tile([128, NST, 8, 65], BF16, tag="v", name="v")
        nc.gpsimd.memset(v_sb[:, :, :, 64:65], 1.0)

        # x chunks per 512-col range
        x_chunks = {"q": [None] * 4, "k": [None] * 4, "v": [None] * 4}

        def emit_x_chunk(which, r, split=False):
            src = {"q": xq_r, "k": xk_r, "v": xv_r}[which]
            t = xt.tile([128, 8, 512], BF16, tag=f"x{which}", name=f"x{which}")
            if split:  # startup path: halves land (and unblock) sooner
                nc.sync.dma_start(t[:, 0:4, :], src[:, 0:4, 512 * r : 512 * r + 512])
                nc.sync.dma_start(t[:, 4:8, :], src[:, 4:8, 512 * r : 512 * r + 512])
            else:
                nc.sync.dma_start(t[:], src[:, :, 512 * r : 512 * r + 512])
            x_chunks[which][r] = t

        # persistent projections
        qT_sb = [qk.tile([128, S], BF16, tag=f"qT{p}", name=f"qT{p}") for p in range(NPAIR)]
        kT_sb = [qk.tile([128, S], BF16, tag=f"kT{p}", name=f"kT{p}") for p in range(NPAIR)]

        # --- projection tile emitters (each = one p1 grab, 16 matmuls) ---
        def emit_qk_tile(which, r, mpair):
            t = p1.tile([128, 2, 512], F32, tag="u", name="pj")
            xc = x_chunks[which][r]
            w_sb = wq_sb if which == "q" else wk_sb
            dst = qT_sb if which == "q" else kT_sb
            for half in range(2):
                m = 2 * mpair + half
                for ic in range(8):
                    nc.tensor.matmul(
                        t[:, half, :],
                        w_sb[:, ic, 128 * m : 128 * m + 128],
                        xc[:, ic, :],
                        start=(ic == 0),
                        stop=(ic == 7),
                    )
            for half in range(2):
                m = 2 * mpair + half
                if which == "q":
                    nc.vector.tensor_scalar_add(
                        dst[m][:, 512 * r : 512 * r + 512], t[:, half, :],
                        bq_sb[:, m : m + 1],
                    )
                else:
                    nc.vector.tensor_copy(
                        dst[m][:, 512 * r : 512 * r + 512], t[:, half, :]
                    )

        def emit_v_tile(r, half_idx):
            t = p1.tile([128, 2, 8, 64], F32, tag="u", name="pv")
            xc = x_chunks["v"][r]
            for half in range(2):
                st_local = 2 * half_idx + half
                for ic in range(8):
                    nc.tensor.matmul(
                        t[:, half, :, :],
                        xc[:, ic, 128 * st_local : 128 * st_local + 128],
                        wv_sb[:, ic, :],
                        start=(ic == 0),
                        stop=(ic == 7),
                    )
            for half in range(2):
                st = 4 * r + 2 * half_idx + half
                nc.vector.tensor_copy(v_sb[:, st, :, 0:64], t[:, half, :, :])

        # --- y projection tile emitter (one 128-row q stripe, full H) ---
        o_tiles = {}  # (slab, pair) -> o_sb

        def emit_y_tile(s, st):
            t = p1.tile([128, 2, 512], F32, tag="u", name="yt")
            for jsl in range(2):
                for p in range(NPAIR):
                    nc.tensor.matmul(
                        t[:, jsl, :],
                        o_tiles[(s, p)][:, 128 * st : 128 * st + 128],
                        wo_sb[:, p, 512 * jsl : 512 * jsl + 512],
                        start=(p == 0),
                        stop=(p == NPAIR - 1),
                    )
            ysb = ys.tile([128, 2, 512], BF16, tag="ys", name="ysb")
            nc.vector.tensor_copy(ysb[:], t[:])
            srow = 512 * s + 128 * st
            nc.sync.dma_start(yv[srow : srow + 128, :, :], ysb[:])

        # --- attention emitters ---
        # unit = (slab, pair, head, g); group g covers k-tiles (2g, 2g+1)
        ot_ps = {}  # (pair, head) -> current PV psum tile

        def emit_scores(s, p, h, g):
            q0 = 512 * s
            r0 = 64 * h
            sc = p1.tile([128, 2, 512], F32, tag="u", name="sc")
            pb_t = pb.tile([128, 2, 512], BF16, tag="pb", name="pb")
            offs = []
            for j, kt in enumerate((2 * g, 2 * g + 1)):
                o = 128 * (kt - 4 * s) if kt >= 4 * s else 0
                offs.append(o)
                nc.tensor.matmul(
                    sc[:, j, o:512],
                    kT_sb[p][r0 : r0 + 64, 128 * kt : 128 * kt + 128],
                    qT_sb[p][r0 : r0 + 64, q0 + o : q0 + 512],
                    start=True,
                    stop=not general_mask,
                )
                if general_mask:
                    nc.tensor.matmul(
                        sc[:, j, o:512],
                        mb_sb[0:1, 128 * kt : 128 * kt + 128],
                        ones_row[0:1, 0 : 512 - o],
                        start=False,
                        stop=True,
                    )
            if offs[0] == 0 and offs[1] == 0:
                nc.scalar.activation(
                    pb_t[:, :, :], sc[:, :, :], AF.Exp,
                    bias=zeros_col[:, 0:1], scale=0.125,
                )
            else:
                for j, o in enumerate(offs):
                    nc.scalar.activation(
                        pb_t[:, j, o:512], sc[:, j, o:512], AF.Exp,
                        bias=zeros_col[:, 0:1], scale=0.125,
                    )
            for j, kt in enumerate((2 * g, 2 * g + 1)):
                if kt >= 4 * s:  # diagonal block: mask the 128-wide triangle band
                    o = offs[j]
                    nc.vector.tensor_tensor(
                        pb_t[:, j, o : o + 128],
                        pb_t[:, j, o : o + 128],
                        tri_sb[:, 0:128],
                        ALU.mult,
                    )
            return pb_t

        def emit_pv(s, p, h, g, pb_t):
            n_kt = 4 * (s + 1)
            if g == 0:
                ot_ps[(p, h)] = p2.tile([128, 512], F32, tag="ot", name="ot")
            acc = ot_ps[(p, h)]
            for j, kt in enumerate((2 * g, 2 * g + 1)):
                o = 128 * (kt - 4 * s) if kt >= 4 * s else 0
                nc.tensor.matmul(
                    acc[0:65, o:512],
                    v_sb[:, kt, 2 * p + h, 0:65],
                    pb_t[:, j, o:512],
                    start=(kt == 0),
                    stop=(kt == n_kt - 1),
                )

        # Normalization, fully per-pair (spread evenly, no DVE lumps):
        # evacuate both PV banks to SBUF (frees PSUM early), gather the two
        # denominator rows into a [33,512] tile at legal bases {0,32}, compute
        # 1/L = exp(-ln L) on the Scalar engine (ln and exp share one
        # activation table, so no table thrash; ~0.65us per op vs a 4us DVE
        # reciprocal), broadcast each row with a K=1 ones matmul on the PE
        # into p2 PSUM, and finish with two DVE multiplies.
        norm_q = []  # deferred finish thunks: [countdown, thunk]
        o_parts = {}  # (s, p) -> (osfA, osfB) awaiting finish_norm

        def emit_norm(s, p):
            accA = ot_ps[(p, 0)]
            accB = ot_ps[(p, 1)]
            osfA = ev.tile([64, 512], F32, tag="osf", name="osfA", bufs=10)
            osfB = ev.tile([64, 512], F32, tag="osf", name="osfB", bufs=10)
            nc.vector.tensor_copy(osfA[:], accA[0:64, :])
            nc.vector.tensor_copy(osfB[:], accB[0:64, :])
            lt = ev.tile([33, 512], F32, tag="Lt", name="lt", bufs=4)
            nc.gpsimd.memset(lt[:], 1.0)
            nc.vector.tensor_copy(lt[0:1, :], accA[64:65, :])
            nc.vector.tensor_copy(lt[32:33, :], accB[64:65, :])
            lnt = ev.tile([33, 512], F32, tag="Lt", name="lnt", bufs=4)
            nc.scalar.activation(
                lnt[:], lt[:], AF.Ln, bias=zeros_col[0:33, 0:1], scale=1.0
            )
            rb = ev.tile([33, 512], BF16, tag="rb", name="rb", bufs=4)
            nc.scalar.activation(
                rb[:], lnt[:], AF.Exp, bias=zeros_col[0:33, 0:1], scale=-1.0
            )
            o_parts[(s, p)] = (osfA, osfB)
            # The PE broadcast + DVE multiply wait on the ACT ln/exp chain, so
            # defer them a few units: by then the reciprocal row is ready and
            # the in-order PE queue never stalls on it.
            norm_q.append([3, lambda: finish_norm(s, p, rb)])

        def finish_norm(s, p, rb):
            osfA, osfB = o_parts.pop((s, p))
            rxA = p2.tile([64, 512], F32, tag="ot", name="rxA")
            nc.tensor.matmul(
                rxA[:], ones_t[0:1, :], rb[0:1, :], start=True, stop=True
            )
            rxB = p2.tile([64, 512], F32, tag="ot", name="rxB")
            nc.tensor.matmul(
                rxB[:], ones_t[32:33, :], rb[32:33, :], start=True, stop=True
            )
            o_sb = ot.tile([128, 512], BF16, tag="ot", name="osb")
            nc.vector.tensor_tensor(o_sb[0:64, :], osfA[:, :], rxA[:, :], ALU.mult)
            nc.vector.tensor_tensor(o_sb[64:128, :], osfB[:, :], rxB[:, :], ALU.mult)
            o_tiles[(s, p)] = o_sb

        # --- emission schedule ---
        # DMA order: interleave weights with the range-0 x chunks so the first
        # projection tile's inputs arrive as early as possible.
        wq_r = wqT.rearrange("(po pi) o -> pi po o", pi=128)
        nc.sync.dma_start(wq_sb[:, :, 0:256], wq_r[:, :, 0:256])
        emit_x_chunk("q", 0, split=True)
        nc.sync.dma_start(wq_sb[:, :, 256:512], wq_r[:, :, 256:512])
        nc.sync.dma_start(bq_sb[:], bqc[:, :])
        wk_r = wkT.rearrange("(po pi) o -> pi po o", pi=128)
        nc.sync.dma_start(wk_sb[:, :, 0:256], wk_r[:, :, 0:256])
        nc.sync.dma_start(wk_sb[:, :, 256:512], wk_r[:, :, 256:512])
        emit_x_chunk("k", 0, split=True)
        nc.sync.dma_start(wv_sb[:], wvT.rearrange("(po pi) o -> pi po o", pi=128))
        emit_x_chunk("v", 0)
        nc.sync.dma_start(tri_sb[:], tri[:, :])
        nc.sync.dma_start(wo_sb[:], woT.rearrange("(po pi) j -> pi po j", pi=128))
        for r in range(1, 4):
            for which in ("q", "k", "v"):
                emit_x_chunk(which, r)

        for mpair in range(2):
            emit_qk_tile("q", 0, mpair)
        for mpair in range(2):
            emit_qk_tile("k", 0, mpair)
        for half_idx in range(2):
            emit_v_tile(0, half_idx)

        # Filler distribution: k(s)/v(s) tiles land at the START of slab s
        # (just-in-time for k-tiles 4s..4s+3, first consumed at unit 4s) so
        # each slab opens with a long dense PE burst; q(s+1) spreads across
        # slab s; y tiles backfill the ACT-paced late slabs.
        front_fillers = {
            1: [lambda: emit_qk_tile("k", 1, 0), lambda: emit_qk_tile("k", 1, 1),
                lambda: emit_v_tile(1, 0), lambda: emit_v_tile(1, 1)],
            2: [lambda: emit_qk_tile("k", 2, 0), lambda: emit_qk_tile("k", 2, 1),
                lambda: emit_v_tile(2, 0), lambda: emit_v_tile(2, 1)],
            3: [lambda: emit_qk_tile("k", 3, 0), lambda: emit_qk_tile("k", 3, 1),
                lambda: emit_v_tile(3, 0), lambda: emit_v_tile(3, 1)],
        }
        spread_fillers = {
            0: [lambda: emit_qk_tile("q", 1, 0), lambda: emit_qk_tile("q", 1, 1)],
            1: [lambda: emit_qk_tile("q", 2, 0), lambda: emit_qk_tile("q", 2, 1)],
            2: [lambda: emit_qk_tile("q", 3, 0), lambda: emit_qk_tile("q", 3, 1)]
               + [lambda st=st: emit_y_tile(0, st) for st in range(4)],
            3: [lambda st=st: emit_y_tile(1, st) for st in range(4)]
               + [lambda st=st: emit_y_tile(2, st) for st in range(4)],
        }

        pending = []  # up to 2 units (s, p, h, g, pb_t) awaiting PV (lag-2)
        for s in range(NSLAB):
            front = list(front_fillers.get(s, []))
            fillers = list(spread_fillers.get(s, []))
            units = [
                (p, h, g)
                for p in range(NPAIR)
                for g in range(2 * (s + 1))
                for h in range(2)
            ]
            every = max(1, -(-len(units) // max(1, len(fillers))))
            fi = 0
            for i, (p, h, g) in enumerate(units):
                pb_t = emit_scores(s, p, h, g)
                if i < len(front):
                    front[i]()
                elif fi < len(fillers) and i % every == every - 1:
                    fillers[fi]()
                    fi += 1
                if len(pending) >= 3:
                    ps_, pp_, ph_, pg_, ppb = pending.pop(0)
                    emit_pv(ps_, pp_, ph_, pg_, ppb)
                    if ph_ == 1 and pg_ == 2 * (ps_ + 1) - 1:
                        emit_norm(ps_, pp_)
                for item in norm_q:
                    item[0] -= 1
                while norm_q and norm_q[0][0] <= 0:
                    norm_q.pop(0)[1]()
                pending.append((s, p, h, g, pb_t))
            while fi < len(fillers):
                fillers[fi]()
                fi += 1

        while pending:
            ps_, pp_, ph_, pg_, ppb = pending.pop(0)
            emit_pv(ps_, pp_, ph_, pg_, ppb)
            if ph_ == 1 and pg_ == 2 * (ps_ + 1) - 1:
                emit_norm(ps_, pp_)
        while norm_q:
            norm_q.pop(0)[1]()
        for st in range(4):
            emit_y_tile(3, st)

    nc.compile()
    _BUILT[general_mask] = nc
    return nc


def _prep_core(query, key, value, mask, Wq, Wk, Wv, Wo, bq, core):
    b, hg = core // 2, core % 2
    o_sl = slice(hg * O, hg * O + O)
    bf = ml_dtypes.bfloat16

    d = np.arange(128)[None, :]
    kk = np.arange(128)[:, None]
    tri = (d >= kk).astype(np.float32)

    mrow = np.where(mask[b] > 0, 0.0, MASK_FILL).astype(np.float32)

    return {
        "xqT": np.ascontiguousarray(query[b].T).astype(bf),
        "xkT": np.ascontiguousarray(key[b].T).astype(bf),
        "xvT": np.ascontiguousarray(value[b].T).astype(bf),
        "wqT": np.ascontiguousarray(Wq[o_sl].T).astype(bf),
        "wkT": np.ascontiguousarray(Wk[o_sl].T).astype(bf),
        "wvT": np.ascontiguousarray(Wv[o_sl].T).astype(bf),
        "woT": np.ascontiguousarray(Wo[:, o_sl].T).astype(bf),
        "bqc": np.ascontiguousarray(bq[o_sl].reshape(4, 128).T).astype(np.float32),
        "tri": tri.astype(bf),
        "mb": mrow.reshape(1, S).astype(bf),
    }


def kernel(query, key, value, mask, Wq, bq, Wk, bk, Wv, bv, Wo, bo, _trace=False):
    general_mask = bool(np.any(np.asarray(mask) <= 0))
    nc = _build(general_mask)
    in_maps = [
        _prep_core(query, key, value, mask, Wq, Wk, Wv, Wo, bq, c)
        for c in range(8)
    ]
    res = run_bass_kernel_spmd(
        nc, in_maps, core_ids=list(range(8)), trace=_trace,
        trace_cores=list(range(8)) if _trace else None,
    )
    parts = np.stack(
        [np.asarray(res.results[c]["y"], dtype=np.float32) for c in range(8)]
    )  # [8, S, H]
    # V bias folded through the output projection (exact: sum_k probs = 1
    # after normalization); K bias dropped (softmax shift-invariant).
    bias = np.asarray(bo) + np.asarray(bv) @ np.asarray(Wo).T
    out = parts[0::2] + parts[1::2] + bias[None, None, :]
    if _trace:
        kernel.last_results = res
    return out.astype(np.float32)
